# revision 18
# baseline (speedup 1.0000x reference)
"""BitAttention (BitNet-style ternary-quantized attention) on 8 Trainium2
NeuronCores.

Sharding: tensor-parallel across heads. 16 heads / 8 cores = 2 heads per
core. Each core computes q/k/v projections for its 2 heads (output-dim
shard), causal attention for those heads, and a partial out-projection
(input-dim shard of wo). Host sums the 8 partial outputs (the all-reduce
of the hint, done at unshard time).

Weight quantization sign(w) * mean(|w|) is separable: the +-1 sign
matrices are exact in bf16 and become matmul operands; the four scalar
scales are folded into the softmax exp scale and the output scale, both
applied on-device in fp32 via tiny input tensors.

Layouts (per core):
  xT   [D, B*T]  x transposed (host), bf16 - moving operand of q/k proj,
                 stationary of v proj.
  q^T,k^T kept [head_dim, tok] in SBUF; V kept [tok, head_dim];
  scores computed transposed S^T = [k-tok, q-tok] so that
  P^T = exp(S^T) feeds the y^T matmul directly (no on-chip transposes
  anywhere). Softmax denominator accumulated on DVE, reduced across
  partitions on GpSimd, broadcast back via a rank-1 matmul.
"""

import numpy as np
import ml_dtypes

B, T, D, H = 2, 2048, 2048, 16
HD = 128  # head dim
NCORES = 8
HPC = H // NCORES  # heads per core = 2
HDC = HPC * HD  # per-core projection width = 256
BT = B * T  # 4096

QT = 512  # q-tile (free dim of S^T / y^T matmuls)
KB = 128  # k-block (partition dim of S^T)

_cache = {}


def _build_nc():
    import concourse.tile as tile
    from concourse import bacc, mybir

    f32 = mybir.dt.float32
    bf16 = mybir.dt.bfloat16
    f8 = mybir.dt.float8e4
    DR = mybir.MatmulPerfMode.DoubleRow

    nc = bacc.Bacc("TRN2", target_bir_lowering=False, debug=False,
                   num_devices=NCORES)

    xT = nc.dram_tensor("xT", [D, BT], bf16, kind="ExternalInput").ap()
    wqT = nc.dram_tensor("wqT", [D, HDC], bf16, kind="ExternalInput").ap()
    wkT = nc.dram_tensor("wkT", [D, HDC], bf16, kind="ExternalInput").ap()
    wvT = nc.dram_tensor("wvT", [D, HDC], bf16, kind="ExternalInput").ap()
    woT = nc.dram_tensor("woT", [HDC, D], bf16, kind="ExternalInput").ap()
    # scal_qk: [128,1] filled with s_q*s_k/sqrt(HD) (folded into q^T)
    # scal_vo: [1,1] filled with 1/(s_v*s_o) (folded into softmax denom)
    scal_qk = nc.dram_tensor("scal_qk", [128, 1], f32, kind="ExternalInput").ap()
    scal_vo = nc.dram_tensor("scal_vo", [1, 1], f32, kind="ExternalInput").ap()
    out = nc.dram_tensor("out", [BT, D], bf16, kind="ExternalOutput").ap()

    with tile.TileContext(nc) as tc:
        with (
            tc.tile_pool(name="singles", bufs=1) as singles,
            tc.tile_pool(name="xstream", bufs=2) as xstream,
            tc.tile_pool(name="work", bufs=6) as work,
            tc.tile_pool(name="dwork", bufs=3) as dwork,
            tc.tile_pool(name="outsb", bufs=6) as outsb,
        ):
            # ---- persistent SBUF tensors -------------------------------
            wq_sb = singles.tile([128, D // 128, HDC], bf16, tag="wq")
            wk_sb = singles.tile([128, D // 128, HDC], bf16, tag="wk")
            wv_sb = singles.tile([128, D // 128, HDC], bf16, tag="wv")
            wo_sb = singles.tile([128, HPC, D], bf16, tag="wo")
            sqk_sb = singles.tile([128, 1], f32, tag="sqk")
            svo_sb = singles.tile([1, 1], f32, tag="svo")
            ones_col = singles.tile([128, 1], bf16, tag="ones")
            nc.vector.memset(ones_col, 1.0)
            # 0/1 causal masks for the two diagonal pair offsets
            from concourse.mybir import AluOpType as _Alu
            mask_sb = []
            for mi in range(QT // KB):
                mk = singles.tile([128, QT], bf16, tag=f"mask{mi}",
                                  name=f"mask{mi}")
                nc.gpsimd.memset(mk, 1.0)
                nc.gpsimd.affine_select(
                    out=mk, in_=mk,
                    pattern=[[1, QT]],
                    channel_multiplier=-1,
                    base=-mi * KB,
                    compare_op=_Alu.is_ge,
                    fill=0.0,
                )
                mask_sb.append(mk)
            # per-batch / per-unit splits so later stages can start as soon
            # as their slice of the data is ready (Tile deps are per-tile)
            qT_sb = [singles.tile([128, HPC, T], bf16, tag=f"qT{b}", name=f"qT{b}")
                     for b in range(B)]
            kT_sb = [singles.tile([128, HPC, T], bf16, tag=f"kT{b}", name=f"kT{b}")
                     for b in range(B)]
            v_sb = [singles.tile([128, T // 128, HDC], bf16, tag=f"v{b}", name=f"v{b}")
                    for b in range(B)]
            yT_sb = [[singles.tile([128, T], bf16, tag=f"yT{b}{hh}", name=f"yT{b}{hh}")
                      for hh in range(HPC)] for b in range(B)]

            nc.gpsimd.dma_start(out=wq_sb, in_=wqT.rearrange("(c p) m -> p c m", p=128))
            nc.gpsimd.dma_start(out=wk_sb, in_=wkT.rearrange("(c p) m -> p c m", p=128))
            nc.gpsimd.dma_start(out=wv_sb, in_=wvT.rearrange("(c p) m -> p c m", p=128))
            nc.gpsimd.dma_start(out=wo_sb, in_=woT.rearrange("(c p) m -> p c m", p=128))
            nc.gpsimd.dma_start(out=sqk_sb, in_=scal_qk)
            nc.gpsimd.dma_start(out=svo_sb, in_=scal_vo)

            # ---- single whole-kernel PSUM pool --------------------------
            # tags: "s" (3 banks: stage-A accum chains + attention S tiles),
            #       "y" (2 banks: attention y accumulators),
            #       "mix" (3 banks: A chains early / softmax d + out-proj o)
            # Total 8 banks, no pool boundaries, so stages overlap freely.
            from concourse.mybir import ActivationFunctionType as AF

            NKK = D // 128  # 16 contraction chunks
            xTr = xT.rearrange("(c p) n -> p c n", p=128)
            LOOKAHEAD = 3  # k-blocks of S issued ahead of their d/y matmuls

            with tc.tile_pool(name="ps", bufs=1, space="PSUM") as psP:

                def emit_A(j):
                    b, jb = divmod(j, T // QT)
                    xt = xstream.tile([128, NKK, QT], bf16, tag="xt",
                                      name="xt")
                    nc.sync.dma_start(out=xt, in_=xTr[:, :, j * QT:(j + 1) * QT])
                    # q^T, k^T : [head_dim part, tok free]
                    for (wsb, dst) in ((wq_sb, qT_sb[b]), (wk_sb, kT_sb[b])):
                        for h2 in range(HPC):
                            ps = psP.tile([128, QT], f32, tag="mix", bufs=3,
                                          name="psA")
                            for kk in range(NKK):
                                nc.tensor.matmul(
                                    ps,
                                    lhsT=wsb[:, kk, h2 * HD:(h2 + 1) * HD],
                                    rhs=xt[:, kk, :],
                                    start=(kk == 0), stop=(kk == NKK - 1),
                                )
                            if wsb is wq_sb:
                                # fold exp scale s_q*s_k/sqrt(hd) into q^T
                                nc.vector.tensor_scalar_mul(
                                    dst[:, h2, jb * QT:(jb + 1) * QT], ps, sqk_sb)
                            else:
                                nc.any.tensor_copy(
                                    dst[:, h2, jb * QT:(jb + 1) * QT], ps)
                    # v : [tok part, head_dim free]
                    for mm in range(QT // 128):  # 4 token chunks of 128
                        ps = psP.tile([128, HDC], f32, tag="mix", bufs=3,
                                      name="psV")
                        for kk in range(NKK):
                            nc.tensor.matmul(
                                ps,
                                lhsT=xt[:, kk, mm * 128:(mm + 1) * 128],
                                rhs=wv_sb[:, kk, :],
                                start=(kk == 0), stop=(kk == NKK - 1),
                            )
                        nc.any.tensor_copy(v_sb[b][:, jb * 4 + mm, :], ps)

                def emit_B(b, hh, iq):
                    nkb = (iq + 1) * (QT // KB)
                    y_ps = psP.tile([128, QT], f32, tag="y", bufs=2, name="psYt")
                    d_ps = psP.tile([1, QT], f32, tag="mix", bufs=3, name="psDt")

                    def s_block(kb):
                        s_ps = psP.tile([128, QT], f32, tag="s", bufs=3,
                                        name="psSt")
                        nc.tensor.matmul(
                            s_ps,
                            lhsT=kT_sb[b][:, hh, kb * KB:(kb + 1) * KB],
                            rhs=qT_sb[b][:, hh, iq * QT:(iq + 1) * QT],
                            start=True, stop=True,
                        )
                        pt = work.tile([128, QT], bf16, tag="pt", name="pt")
                        nc.scalar.activation(pt, s_ps, AF.Exp)
                        if kb >= iq * (QT // KB):
                            # block touches the diagonal: zero k>q
                            nc.vector.tensor_mul(
                                pt, pt, mask_sb[kb - iq * (QT // KB)])
                        return pt

                    pts = [s_block(kb) for kb in range(min(LOOKAHEAD, nkb))]
                    for kb in range(nkb):
                        pt = pts[kb]
                        if kb + LOOKAHEAD < nkb:
                            pts.append(s_block(kb + LOOKAHEAD))
                        # d[q] += sum_k pt[k, q] (rank-1 on PE)
                        nc.tensor.matmul(
                            d_ps, lhsT=ones_col, rhs=pt,
                            start=(kb == 0), stop=(kb == nkb - 1),
                        )
                        nc.tensor.matmul(
                            y_ps,
                            lhsT=v_sb[b][:, kb, hh * HD:(hh + 1) * HD],
                            rhs=pt,
                            start=(kb == 0), stop=(kb == nkb - 1),
                        )
                    # softmax denominator + normalization (DVE/GpSimd only)
                    d_sb = dwork.tile([1, QT], f32, tag="dsb", name="dsb")
                    nc.vector.tensor_scalar_mul(d_sb, d_ps, svo_sb)
                    dr = dwork.tile([1, QT], f32, tag="dr", name="dr")
                    nc.vector.reciprocal_approx_fast(dr, d_sb)
                    r_sb = dwork.tile([128, QT], f32, tag="rsb", name="rsb")
                    nc.gpsimd.partition_broadcast(r_sb, dr)
                    nc.vector.tensor_mul(
                        yT_sb[b][hh][:, iq * QT:(iq + 1) * QT], y_ps, r_sb)

                def emit_C(b, mb):
                    for n in range(D // QT):  # 4 output column tiles
                        ps = psP.tile([128, QT], f32, tag="mix", bufs=3,
                                      name="psOt")
                        for kk in range(HPC):
                            nc.tensor.matmul(
                                ps,
                                lhsT=yT_sb[b][kk][:, mb * 128:(mb + 1) * 128],
                                rhs=wo_sb[:, kk, n * QT:(n + 1) * QT],
                                start=(kk == 0), stop=(kk == HPC - 1),
                            )
                        o_sb = outsb.tile([128, QT], bf16, tag="osb", name="osb")
                        if (mb + n) % 2 == 0:
                            nc.vector.tensor_copy(o_sb, ps)
                        else:
                            nc.scalar.copy(o_sb, ps)
                        m = b * (T // 128) + mb
                        nc.sync.dma_start(
                            out=out[m * 128:(m + 1) * 128, n * QT:(n + 1) * QT],
                            in_=o_sb)

                # emission schedule (iq-major): A(b0) | A(b1)+B(b0) |
                # B(b1)+C(b0)+C(b1,lagged) | C(b1) tail
                for j in range(4):
                    emit_A(j)
                b0_tiles = [(0, hh, iq) for iq in range(T // QT)
                            for hh in range(HPC)]
                for j in range(4, 8):
                    emit_A(j)
                    emit_B(*b0_tiles.pop(0))
                    emit_B(*b0_tiles.pop(0))
                while b0_tiles:
                    emit_B(*b0_tiles.pop(0))
                for iq in range(T // QT):
                    emit_B(1, 0, iq)
                    emit_B(1, 1, iq)
                    for mb in range(4 * iq, 4 * iq + 4):
                        emit_C(0, mb)
                    if iq >= 1:
                        for mb in range(4 * (iq - 1), 4 * (iq - 1) + 4):
                            emit_C(1, mb)
                for mb in range(12, 16):
                    emit_C(1, mb)

    nc.compile()
    return nc


def kernel(x, wq, wk, wv, wo):
    import concourse.bass_utils as bass_utils

    x = np.asarray(x, dtype=np.float32)
    bf16 = ml_dtypes.bfloat16

    if "nc" not in _cache:
        _cache["nc"] = _build_nc()
    nc = _cache["nc"]

    scales = {}
    signs = {}
    for name, w in (("q", wq), ("k", wk), ("v", wv), ("o", wo)):
        w = np.asarray(w, dtype=np.float32)
        scales[name] = max(np.mean(np.abs(w)), 1e-5)
        signs[name] = np.sign(w)

    s_qk = np.float32(scales["q"] * scales["k"] / np.sqrt(HD))
    s_vo = np.float32(1.0 / (scales["v"] * scales["o"]))

    xT = np.ascontiguousarray(x.reshape(BT, D).T).astype(bf16)
    scal_qk = np.full((128, 1), s_qk, dtype=np.float32)
    scal_vo = np.full((1, 1), s_vo, dtype=np.float32)

    in_maps = []
    for c in range(NCORES):
        sl = slice(c * HDC, (c + 1) * HDC)
        in_maps.append({
            "xT": xT,
            "wqT": np.ascontiguousarray(signs["q"][sl, :].T).astype(bf16),
            "wkT": np.ascontiguousarray(signs["k"][sl, :].T).astype(bf16),
            "wvT": np.ascontiguousarray(signs["v"][sl, :].T).astype(bf16),
            "woT": np.ascontiguousarray(signs["o"][:, sl].T).astype(bf16),
            "scal_qk": scal_qk,
            "scal_vo": scal_vo,
        })

    res = bass_utils.run_bass_kernel_spmd(nc, in_maps,
                                          core_ids=list(range(NCORES)),
                                          **_cache.get("run_kwargs", {}))
    _cache["last_result"] = res

    acc = np.zeros((BT, D), dtype=np.float32)
    for r in res.results:
        acc += np.asarray(r["out"], dtype=np.float32)
    return acc.reshape(B, T, D)


# revision 31
# speedup vs baseline: 1.0698x; 1.0698x over previous
"""BitAttention (BitNet-style ternary-quantized attention) on 8 Trainium2
NeuronCores.

Sharding: tensor-parallel across heads. 16 heads / 8 cores = 2 heads per
core. Each core computes q/k/v projections for its 2 heads (output-dim
shard), causal attention for those heads, and a partial out-projection
(input-dim shard of wo). Host sums the 8 partial outputs (the all-reduce
of the hint, done at unshard time).

Weight quantization sign(w) * mean(|w|) is separable: the +-1 sign
matrices are exact in bf16 and become matmul operands; the four scalar
scales are folded into the softmax exp scale and the output scale, both
applied on-device in fp32 via tiny input tensors.

Layouts (per core):
  xT   [D, B*T]  x transposed (host), bf16 - moving operand of q/k proj,
                 stationary of v proj.
  q^T,k^T kept [head_dim, tok] in SBUF; V kept [tok, head_dim];
  scores computed transposed S^T = [k-tok, q-tok] so that
  P^T = exp(S^T) feeds the y^T matmul directly (no on-chip transposes
  anywhere). Softmax denominator accumulated on DVE, reduced across
  partitions on GpSimd, broadcast back via a rank-1 matmul.
"""

import numpy as np
import ml_dtypes

B, T, D, H = 2, 2048, 2048, 16
HD = 128  # head dim
NCORES = 8
HPC = H // NCORES  # heads per core = 2
HDC = HPC * HD  # per-core projection width = 256
BT = B * T  # 4096

QT = 512  # q-tile (free dim of S^T / y^T matmuls)
KB = 128  # k-block (partition dim of S^T)

_cache = {}


def _build_nc():
    import concourse.tile as tile
    from concourse import bacc, mybir

    f32 = mybir.dt.float32
    bf16 = mybir.dt.bfloat16
    f8 = mybir.dt.float8e4
    DR = mybir.MatmulPerfMode.DoubleRow

    nc = bacc.Bacc("TRN2", target_bir_lowering=False, debug=False,
                   num_devices=NCORES)

    xT = nc.dram_tensor("xT", [D, BT], bf16, kind="ExternalInput").ap()
    wqT = nc.dram_tensor("wqT", [D, HDC], bf16, kind="ExternalInput").ap()
    wkT = nc.dram_tensor("wkT", [D, HDC], bf16, kind="ExternalInput").ap()
    wvT = nc.dram_tensor("wvT", [D, HDC], bf16, kind="ExternalInput").ap()
    woT = nc.dram_tensor("woT", [HDC, D], bf16, kind="ExternalInput").ap()
    # scal_qk: [128,1] filled with s_q*s_k/sqrt(HD) (folded into q^T)
    # scal_vo: [1,1] filled with 1/(s_v*s_o) (folded into softmax denom)
    scal_qk = nc.dram_tensor("scal_qk", [128, 1], f32, kind="ExternalInput").ap()
    scal_vo = nc.dram_tensor("scal_vo", [1, 1], f32, kind="ExternalInput").ap()
    out = nc.dram_tensor("out", [BT, D], bf16, kind="ExternalOutput").ap()

    with tile.TileContext(nc) as tc:
        with (
            tc.tile_pool(name="singles", bufs=1) as singles,
            tc.tile_pool(name="xstream", bufs=2) as xstream,
            tc.tile_pool(name="work", bufs=20) as work,
            tc.tile_pool(name="dwork", bufs=3) as dwork,
            tc.tile_pool(name="outsb", bufs=6) as outsb,
        ):
            # ---- persistent SBUF tensors -------------------------------
            wq_sb = singles.tile([128, D // 128, HDC], bf16, tag="wq")
            wk_sb = singles.tile([128, D // 128, HDC], bf16, tag="wk")
            wv_sb = singles.tile([128, D // 128, HDC], bf16, tag="wv")
            wo_sb = singles.tile([128, HPC, D], bf16, tag="wo")
            sqk_sb = singles.tile([128, 1], f32, tag="sqk")
            svo_sb = singles.tile([1, 1], f32, tag="svo")
            ones_col = singles.tile([128, 1], bf16, tag="ones")
            nc.vector.memset(ones_col, 1.0)
            # 0/1 causal masks for the two diagonal pair offsets
            from concourse.mybir import AluOpType as _Alu
            mask_sb = []
            for mi in range(QT // KB):
                mk = singles.tile([128, QT], bf16, tag=f"mask{mi}",
                                  name=f"mask{mi}")
                nc.gpsimd.memset(mk, 1.0)
                nc.gpsimd.affine_select(
                    out=mk, in_=mk,
                    pattern=[[1, QT]],
                    channel_multiplier=-1,
                    base=-mi * KB,
                    compare_op=_Alu.is_ge,
                    fill=0.0,
                )
                mask_sb.append(mk)
            # per-batch / per-unit splits so later stages can start as soon
            # as their slice of the data is ready (Tile deps are per-tile)
            qT_sb = [singles.tile([128, HPC, T], bf16, tag=f"qT{b}", name=f"qT{b}")
                     for b in range(B)]
            kT_sb = [singles.tile([128, HPC, T], bf16, tag=f"kT{b}", name=f"kT{b}")
                     for b in range(B)]
            v_sb = [singles.tile([128, T // 128, HDC], bf16, tag=f"v{b}", name=f"v{b}")
                    for b in range(B)]
            yT_sb = [[singles.tile([128, T], bf16, tag=f"yT{b}{hh}", name=f"yT{b}{hh}")
                      for hh in range(HPC)] for b in range(B)]

            nc.gpsimd.dma_start(out=wq_sb, in_=wqT.rearrange("(c p) m -> p c m", p=128))
            nc.gpsimd.dma_start(out=wk_sb, in_=wkT.rearrange("(c p) m -> p c m", p=128))
            nc.gpsimd.dma_start(out=wv_sb, in_=wvT.rearrange("(c p) m -> p c m", p=128))
            nc.gpsimd.dma_start(out=wo_sb, in_=woT.rearrange("(c p) m -> p c m", p=128))
            nc.gpsimd.dma_start(out=sqk_sb, in_=scal_qk)
            nc.gpsimd.dma_start(out=svo_sb, in_=scal_vo)

            # ---- single whole-kernel PSUM pool --------------------------
            # tags: "s" (3 banks: stage-A accum chains + attention S tiles),
            #       "y" (2 banks: attention y accumulators),
            #       "mix" (3 banks: A chains early / softmax d + out-proj o)
            # Total 8 banks, no pool boundaries, so stages overlap freely.
            from concourse.mybir import ActivationFunctionType as AF

            NKK = D // 128  # 16 contraction chunks
            xTr = xT.rearrange("(c p) n -> p c n", p=128)
            LOOKAHEAD = 3  # k-blocks of S issued ahead of their d/y matmuls

            with tc.tile_pool(name="ps", bufs=1, space="PSUM") as psP:

                def emit_A(j):
                    b, jb = divmod(j, T // QT)
                    xt = xstream.tile([128, NKK, QT], bf16, tag="xt",
                                      name="xt")
                    nc.sync.dma_start(out=xt, in_=xTr[:, :, j * QT:(j + 1) * QT])
                    # q^T, k^T : [head_dim part, tok free]
                    for (wsb, dst) in ((wq_sb, qT_sb[b]), (wk_sb, kT_sb[b])):
                        for h2 in range(HPC):
                            ps = psP.tile([128, QT], f32, tag="mix", bufs=3,
                                          name="psA")
                            for kk in range(NKK):
                                nc.tensor.matmul(
                                    ps,
                                    lhsT=wsb[:, kk, h2 * HD:(h2 + 1) * HD],
                                    rhs=xt[:, kk, :],
                                    start=(kk == 0), stop=(kk == NKK - 1),
                                )
                            if wsb is wq_sb:
                                # fold exp scale s_q*s_k/sqrt(hd) into q^T
                                nc.vector.tensor_scalar_mul(
                                    dst[:, h2, jb * QT:(jb + 1) * QT], ps, sqk_sb)
                            else:
                                nc.vector.tensor_copy(
                                    dst[:, h2, jb * QT:(jb + 1) * QT], ps)
                    # v : [tok part, head_dim free]
                    for mm in range(QT // 128):  # 4 token chunks of 128
                        ps = psP.tile([128, HDC], f32, tag="mix", bufs=3,
                                      name="psV")
                        for kk in range(NKK):
                            nc.tensor.matmul(
                                ps,
                                lhsT=xt[:, kk, mm * 128:(mm + 1) * 128],
                                rhs=wv_sb[:, kk, :],
                                start=(kk == 0), stop=(kk == NKK - 1),
                            )
                        nc.vector.tensor_copy(v_sb[b][:, jb * 4 + mm, :], ps)

                pending_B = []

                def flush_B():
                    if pending_B:
                        pending_B.pop(0)()

                def emit_B(b, hh, iq):
                    nkb = (iq + 1) * (QT // KB)
                    y_ps = psP.tile([128, QT], f32, tag="y", bufs=2, name="psYt")
                    d_ps = psP.tile([1, QT], f32, tag="mix", bufs=3, name="psDt")

                    def s_block(kb):
                        s_ps = psP.tile([128, QT], f32, tag="s", bufs=3,
                                        name="psSt")
                        nc.tensor.matmul(
                            s_ps,
                            lhsT=kT_sb[b][:, hh, kb * KB:(kb + 1) * KB],
                            rhs=qT_sb[b][:, hh, iq * QT:(iq + 1) * QT],
                            start=True, stop=True,
                        )
                        pt = work.tile([128, QT], bf16, tag="pt", name="pt")
                        m = kb - iq * (QT // KB)
                        if m < 0:
                            nc.scalar.activation(pt, s_ps, AF.Exp)
                        else:
                            # diagonal block: cols < m*KB are entirely k>q.
                            # Exp only the valid slice; affine_select zeroes
                            # everything with k>q including the stale prefix
                            # (fill overwrites, runs on the idle GpSimd).
                            qv = m * KB
                            nc.scalar.activation(pt[:, qv:], s_ps[:, qv:],
                                                 AF.Exp)
                            nc.gpsimd.affine_select(
                                out=pt, in_=pt,
                                pattern=[[1, QT]],
                                channel_multiplier=-1,
                                base=-qv,
                                compare_op=_Alu.is_ge,
                                fill=0.0,
                            )
                        return pt

                    # k-loop: only S and y matmuls (k/v stationaries
                    # double-buffer cleanly); d-matmuls run as a burst
                    # afterwards with the ones vector loaded once, so PE
                    # streams back-to-back instead of at isolated-MM rate.
                    pts = [s_block(kb) for kb in range(min(LOOKAHEAD, nkb))]
                    # flush the previous tile's d-burst/epilogue here so its
                    # PE work overlaps this tile's Exp stream on ScalarE
                    flush_B()
                    for kb in range(nkb):
                        if kb + LOOKAHEAD < nkb:
                            pts.append(s_block(kb + LOOKAHEAD))
                        nc.tensor.matmul(
                            y_ps,
                            lhsT=v_sb[b][:, kb, hh * HD:(hh + 1) * HD],
                            rhs=pts[kb],
                            start=(kb == 0), stop=(kb == nkb - 1),
                        )

                    def deferred(b=b, hh=hh, iq=iq, nkb=nkb, pts=pts,
                                 y_ps=y_ps, d_ps=d_ps):
                        for kb in range(nkb):
                            # d[q] += sum_k pt[k, q] (rank-1 on PE)
                            nc.tensor.matmul(
                                d_ps, lhsT=ones_col, rhs=pts[kb],
                                start=(kb == 0), stop=(kb == nkb - 1),
                            )
                        # softmax denominator + normalization
                        d_sb = dwork.tile([1, QT], f32, tag="dsb", name="dsb")
                        nc.vector.tensor_scalar_mul(d_sb, d_ps, svo_sb)
                        dr = dwork.tile([1, QT], f32, tag="dr", name="dr")
                        nc.vector.reciprocal_approx_fast(dr, d_sb)
                        r_sb = dwork.tile([128, QT], f32, tag="rsb", name="rsb")
                        nc.gpsimd.partition_broadcast(r_sb, dr)
                        nc.vector.tensor_mul(
                            yT_sb[b][hh][:, iq * QT:(iq + 1) * QT], y_ps, r_sb)

                    pending_B.append(deferred)

                def emit_C(b, mb, alternate=False):
                    for n in range(D // QT):  # 4 output column tiles
                        ps = psP.tile([128, QT], f32, tag="mix", bufs=3,
                                      name="psOt")
                        for kk in range(HPC):
                            nc.tensor.matmul(
                                ps,
                                lhsT=yT_sb[b][kk][:, mb * 128:(mb + 1) * 128],
                                rhs=wo_sb[:, kk, n * QT:(n + 1) * QT],
                                start=(kk == 0), stop=(kk == HPC - 1),
                            )
                        o_sb = outsb.tile([128, QT], bf16, tag="osb", name="osb")
                        if alternate and (mb + n) % 2 == 0:
                            nc.scalar.copy(o_sb, ps)
                        else:
                            nc.vector.tensor_copy(o_sb, ps)
                        m = b * (T // 128) + mb
                        nc.sync.dma_start(
                            out=out[m * 128:(m + 1) * 128, n * QT:(n + 1) * QT],
                            in_=o_sb)

                # emission schedule (iq-major): A(b0) | A(b1)+B(b0) |
                # B(b1)+C(b0)+C(b1,lagged) | C(b1) tail
                # warm the PE clock (HAM) while the first xT block loads
                wsrc = dwork.tile([128, QT], f32, tag="warmsrc", name="warmsrc")
                nc.vector.memset(wsrc, 0.0)
                wm = psP.tile([128, QT], f32, tag="s", bufs=3, name="warm")
                for _ in range(12):
                    nc.tensor.matmul(wm, lhsT=wsrc[:, :128], rhs=wsrc,
                                     start=True, stop=True)
                # zero-init pt slots so stale data is always finite
                for _ in range(20):
                    ptz = work.tile([128, QT], bf16, tag="pt", name="pt")
                    nc.vector.memset(ptz, 0.0)
                for j in range(4):
                    emit_A(j)
                b0_tiles = [(0, hh, iq) for iq in range(T // QT)
                            for hh in range(HPC)]
                for j in range(4, 8):
                    emit_A(j)
                    emit_B(*b0_tiles.pop(0))
                    emit_B(*b0_tiles.pop(0))
                while b0_tiles:
                    emit_B(*b0_tiles.pop(0))
                for iq in range(T // QT):
                    emit_B(1, 0, iq)
                    emit_B(1, 1, iq)
                    for mb in range(4 * iq, 4 * iq + 4):
                        emit_C(0, mb)
                    if iq >= 1:
                        for mb in range(4 * (iq - 1), 4 * (iq - 1) + 4):
                            emit_C(1, mb, alternate=True)
                flush_B()
                for mb in range(12, 16):
                    emit_C(1, mb, alternate=True)

    nc.compile()
    return nc


def kernel(x, wq, wk, wv, wo):
    import concourse.bass_utils as bass_utils

    x = np.asarray(x, dtype=np.float32)
    bf16 = ml_dtypes.bfloat16

    if "nc" not in _cache:
        _cache["nc"] = _build_nc()
    nc = _cache["nc"]

    scales = {}
    signs = {}
    for name, w in (("q", wq), ("k", wk), ("v", wv), ("o", wo)):
        w = np.asarray(w, dtype=np.float32)
        scales[name] = max(np.mean(np.abs(w)), 1e-5)
        signs[name] = np.sign(w)

    s_qk = np.float32(scales["q"] * scales["k"] / np.sqrt(HD))
    s_vo = np.float32(1.0 / (scales["v"] * scales["o"]))

    xT = np.ascontiguousarray(x.reshape(BT, D).T).astype(bf16)
    scal_qk = np.full((128, 1), s_qk, dtype=np.float32)
    scal_vo = np.full((1, 1), s_vo, dtype=np.float32)

    in_maps = []
    for c in range(NCORES):
        sl = slice(c * HDC, (c + 1) * HDC)
        in_maps.append({
            "xT": xT,
            "wqT": np.ascontiguousarray(signs["q"][sl, :].T).astype(bf16),
            "wkT": np.ascontiguousarray(signs["k"][sl, :].T).astype(bf16),
            "wvT": np.ascontiguousarray(signs["v"][sl, :].T).astype(bf16),
            "woT": np.ascontiguousarray(signs["o"][:, sl].T).astype(bf16),
            "scal_qk": scal_qk,
            "scal_vo": scal_vo,
        })

    res = bass_utils.run_bass_kernel_spmd(nc, in_maps,
                                          core_ids=list(range(NCORES)),
                                          **_cache.get("run_kwargs", {}))
    _cache["last_result"] = res

    acc = np.zeros((BT, D), dtype=np.float32)
    for r in res.results:
        acc += np.asarray(r["out"], dtype=np.float32)
    return acc.reshape(B, T, D)
